# revision 29
# baseline (speedup 1.0000x reference)
"""Trainium2 Bass kernel for nn_Discriminator: 5-layer GRU stack + output projection.

Strategy
--------
Pure data parallel over batch: 1024 batch rows -> 8 cores x 128. Each core runs
the full T=512 sequential scan on its shard.

Mapping (per core):
- Feature-major layout: every on-chip tensor is [feature_partitions, batch=128].
- Layer wavefront: at tick tau, layer l processes timestep (tau - l). All five
  layers' gate math is packed into shared instructions across partitions
  (state rows: l0 0:32, l1 32:48, l2 48:56, l3 56:72, l4 72:104).
- Sigmoid-space state: g = (h+1)/2, so tanh(x) = 2*sigmoid(2x)-1 folds into the
  same sigmoid ACT table, and the update stays a lerp: g' = zc*m + (1-zc)*g
  with zc = 1-z (computed directly by negating the z-gate weights).
- All weight scaling (x2 / x4), bias terms, and h=2g-1 corrections are folded
  into packed weight matrices host-side; biases ride on a constant "ones" row
  of the state tile (row 104), so matmuls produce fully-biased gate inputs.
- Input X is transposed host-side to [T, 64, B] so per-tick x_t tiles are
  direct DMA loads; output is produced as [T, 64, B] and untransposed host-side.
"""

import numpy as np

D = 64
T_FULL = 512
BZ = 1024
NCORES = 8
BC = BZ // NCORES  # 128 batch per core
H = [32, 16, 8, 16, 32]
OFFS = [0, 32, 48, 56, 72]  # state row offset per layer
SH = 104  # sum(H)
SA = 105  # state rows + ones row


def _build_weights(inp):
    """Fold the reference GRU weights into the packed kernel matrices.

    Kernel math per layer l at one tick (all in sigmoid space g=(h+1)/2):
      pR = WR.T @ g_aug (+ Wir0 @ x for l=0 rows)        ; r  = sigmoid(pR)
      pZ = WZ.T @ g_aug (+ -Wiz0 @ x for l=0 rows)       ; zc = sigmoid(pZ) = 1-z
      pN = WN.T @ g_aug (+ 2*Win0 @ x for l=0 rows)      ; (= 2*inn + 2*b_in)
      pHN = WHN.T @ g_aug                                ; (= 2*hn)
      m  = sigmoid(pN + r*pHN)   (= (tanh(nin)+1)/2)
      g' = zc*m + (1-zc)*g
    """
    f32 = np.float32
    WR = np.zeros((SA, SH), f32)
    WZ = np.zeros((SA, SH), f32)
    WN = np.zeros((SA, SH), f32)
    WHN = np.zeros((SA, SH), f32)
    # x-side weights, zero-padded to the full 104 gate rows so the hoisted
    # x-matmuls initialize the whole psum bank (clean has_written bits)
    W0 = np.zeros((D, 3 * SH), f32)
    for l in range(5):
        dh = H[l]
        o = OFFS[l]
        w_ih = np.asarray(inp[f"w_ih_{l}"], f32)  # [3H, di]
        w_hh = np.asarray(inp[f"w_hh_{l}"], f32)  # [3H, dh]
        b_ih = np.asarray(inp[f"b_ih_{l}"], f32)
        b_hh = np.asarray(inp[f"b_hh_{l}"], f32)
        Wir, Wiz, Win = w_ih[:dh], w_ih[dh : 2 * dh], w_ih[2 * dh :]
        Whr, Whz, Whn = w_hh[:dh], w_hh[dh : 2 * dh], w_hh[2 * dh :]
        bir, biz, bin_ = b_ih[:dh], b_ih[dh : 2 * dh], b_ih[2 * dh :]
        bhr, bhz, bhn = b_hh[:dh], b_hh[dh : 2 * dh], b_hh[2 * dh :]

        # recurrent (own-state) parts: h = 2g-1 -> W@h = (2W)@g - rowsum(W)
        WR[o : o + dh, o : o + dh] = (2.0 * Whr).T
        WZ[o : o + dh, o : o + dh] = -(2.0 * Whz).T
        WHN[o : o + dh, o : o + dh] = (4.0 * Whn).T
        r_bias = bir + bhr - Whr.sum(1)
        z_bias = biz + bhz - Whz.sum(1)
        hn_bias = 2.0 * (bhn - Whn.sum(1))
        n_bias = 2.0 * bin_

        if l == 0:
            # x enters raw through W0 (three SH-col blocks: r | z(neg) | n(x2))
            W0[:, 0:32] = Wir.T
            W0[:, SH : SH + 32] = -Wiz.T
            W0[:, 2 * SH : 2 * SH + 32] = (2.0 * Win).T
        else:
            po, pd = OFFS[l - 1], H[l - 1]
            WR[po : po + pd, o : o + dh] = (2.0 * Wir).T
            WZ[po : po + pd, o : o + dh] = -(2.0 * Wiz).T
            WN[po : po + pd, o : o + dh] = (4.0 * Win).T
            r_bias = r_bias - Wir.sum(1)
            z_bias = z_bias - Wiz.sum(1)
            n_bias = n_bias - 2.0 * Win.sum(1)

        WR[SH, o : o + dh] = r_bias
        WZ[SH, o : o + dh] = -z_bias
        WN[SH, o : o + dh] = n_bias
        WHN[SH, o : o + dh] = hn_bias

    w_out = np.asarray(inp["w_out"], f32)  # [64, 32]
    b_out = np.asarray(inp["b_out"], f32)  # [64]
    WY = np.zeros((SA, D), f32)
    WY[OFFS[4] : OFFS[4] + 32, :] = (2.0 * w_out).T
    WY[SH, :] = b_out - w_out.sum(1)

    # constants DMA'd into SBUF (engine APs need 32-aligned partition starts,
    # DMA writes don't — so unaligned initialization goes through these)
    GINIT = np.full((SA, BC), 0.5, f32)
    GINIT[SH] = 1.0
    RST05 = np.full((32, BC), 0.5, f32)
    return {"WR": WR, "WZ": WZ, "WN": WN, "WHN": WHN, "W0": W0, "WY": WY,
            "GINIT": GINIT, "RST05": RST05}


def numpy_forward(inputs, T):
    """Numpy model of the exact kernel math (for validation)."""
    W = _build_weights(inputs)
    X = np.asarray(inputs["imputed_X"], np.float32)[:, :T]  # [B, T, 64]
    B = X.shape[0]
    sig = lambda x: 1.0 / (1.0 + np.exp(-x))
    g = np.full((SA, B), 0.5, np.float32)
    g[SH] = 1.0
    Y = np.zeros((T, D, B), np.float32)
    h4s = {}
    for tau in range(T + 4):
        pR = W["WR"].T @ g
        pZ = W["WZ"].T @ g
        pN = W["WN"].T @ g
        pHN = W["WHN"].T @ g
        if tau < T:
            x = X[:, tau, :].T  # [64, B]
            pR += W["W0"][:, 0:SH].T @ x
            pZ += W["W0"][:, SH : 2 * SH].T @ x
            pN += W["W0"][:, 2 * SH : 3 * SH].T @ x
        r = sig(pR)
        zc = sig(pZ)
        m = sig(pN + r * pHN)
        gn = zc * m + (1.0 - zc) * g[:SH]
        g = np.concatenate([gn, np.ones((1, B), np.float32)], 0)
        for l in range(1, 5):
            if tau == l - 1:
                g[OFFS[l] : OFFS[l] + H[l]] = 0.5
        if tau >= 4:
            Y[tau - 4] = W["WY"].T @ g
    return Y.transpose(2, 0, 1)  # [B, T, 64]


_prog_cache = {}

# scheduling knobs (tuned via the timeline simulator)
_MM_ORDER = "RHZN"  # per-gate matmul emission order on PE
_ACT_ORDER = "rnz"  # sigma emission order on ACT ("n" = tm,v,sigma_n group)
_YCOPY = "scalar"


def _split_excess_waits(nc, limit=1):
    """The walrus build here accepts at most one sync-wait per instruction;
    Tile emits several on barrier drains etc. Split extras onto NoOps."""
    from concourse import mybir

    n_new = 0
    for f in nc.m.functions:
        for bb in f.blocks:
            changed = False
            new_list = []
            for ins in bb.instructions:
                si = ins.sync_info
                if si is not None and si.on_wait and len(si.on_wait) > limit:
                    waits = list(si.on_wait)
                    while len(waits) > limit:
                        chunk, waits = waits[:limit], waits[limit:]
                        nop = mybir.InstNoOp(
                            name=f"{ins.name}-ws{n_new}",
                            engine=ins.engine,
                            sync_info=mybir.SyncInfo(on_wait=chunk, on_update=[]),
                        )
                        new_list.append(nop)
                        n_new += 1
                    ins.sync_info = mybir.SyncInfo(
                        on_wait=list(waits), on_update=list(si.on_update)
                    )
                    changed = True
                new_list.append(ins)
            if changed:
                bb.instructions = new_list
    return n_new


def _build_program(T):
    if T in _prog_cache:
        return _prog_cache[T]
    import concourse.bass as bass
    import concourse.tile as tile
    from concourse.tile import add_dep_helper
    from concourse import mybir

    f32 = mybir.dt.float32
    SIG = mybir.ActivationFunctionType.Sigmoid
    SUB = mybir.AluOpType.subtract
    MUL = mybir.AluOpType.mult

    XC = 32 if T % 32 == 0 else T  # ticks per input chunk
    YC = 4  # ticks per output chunk
    assert T % YC == 0

    nc = bass.Bass(trn_type="TRN2", name=f"gru_wave_{T}")
    XT = nc.dram_tensor("XT", [T, D, BC], f32, kind="ExternalInput")
    dWR = nc.dram_tensor("WR", [SA, SH], f32, kind="ExternalInput")
    dWZ = nc.dram_tensor("WZ", [SA, SH], f32, kind="ExternalInput")
    dWN = nc.dram_tensor("WN", [SA, SH], f32, kind="ExternalInput")
    dWHN = nc.dram_tensor("WHN", [SA, SH], f32, kind="ExternalInput")
    dW0 = nc.dram_tensor("W0", [D, 3 * SH], f32, kind="ExternalInput")
    dWY = nc.dram_tensor("WY", [SA, D], f32, kind="ExternalInput")
    dGINIT = nc.dram_tensor("GINIT", [SA, BC], f32, kind="ExternalInput")
    dRST = nc.dram_tensor("RST05", [32, BC], f32, kind="ExternalInput")
    YT = nc.dram_tensor("YT", [T, D, BC], f32, kind="ExternalOutput")

    NT = T + 4

    with tile.TileContext(nc) as tc:
        with (
            tc.tile_pool(name="consts", bufs=1) as consts,
            tc.tile_pool(name="gpool", bufs=5) as gpool,
            tc.tile_pool(name="xpool", bufs=2) as xpool,
            tc.tile_pool(name="ypool", bufs=2) as ypool,
            tc.tile_pool(name="work", bufs=2) as work,
            tc.tile_pool(name="psum", bufs=2, space="PSUM") as psum,
            tc.tile_pool(name="hnpsum", bufs=1, space="PSUM") as hnpsum,
            tc.tile_pool(name="ypsum", bufs=1, space="PSUM") as ypsum,
        ):
            wr = consts.tile([SA, SH], f32, tag="wr")
            wz = consts.tile([SA, SH], f32, tag="wz")
            wn = consts.tile([SA, SH], f32, tag="wn")
            whn = consts.tile([SA, SH], f32, tag="whn")
            w0 = consts.tile([D, 3 * SH], f32, tag="w0")
            wy = consts.tile([SA, D], f32, tag="wy")
            for sb, dr in ((wr, dWR), (wz, dWZ), (wn, dWN), (whn, dWHN),
                           (w0, dW0), (wy, dWY)):
                nc.sync.dma_start(out=sb[:], in_=dr[:])

            g_init = consts.tile([SA, BC], f32, tag="ginit")
            nc.sync.dma_start(out=g_init[:], in_=dGINIT[:])

            # establish ones-row (row 104) in all 5 physical g slots
            for _ in range(5):
                gw = gpool.tile([SA, BC], f32, tag="g")
                nc.sync.dma_start(out=gw[:], in_=dGINIT[:])

            nchunks = (T + XC - 1) // XC
            xchunks = []

            def load_xchunk(ci):
                t0 = ci * XC
                nt = min(XC, T - t0)
                xc = xpool.tile([D, XC, BC], f32, tag="xc")
                # split into sub-DMAs for queue parallelism
                step = 8
                for s0 in range(0, nt, step):
                    s1 = min(s0 + step, nt)
                    nc.gpsimd.dma_start(
                        xc[:, s0:s1, :],
                        XT[t0 + s0 : t0 + s1].rearrange("t f b -> f t b"),
                    )
                xchunks.append(xc)

            load_xchunk(0)

            handles = {}
            prev = g_init

            def alloc_gates(tau):
                # Allocate tick tau's R/Z/N psum banks and pre-run its x-side
                # matmuls: they depend only on (prefetched) x, so they execute
                # in the previous tick's idle PE window. The main state
                # matmuls later accumulate on top (start=False).
                pRt = psum.tile([SH, BC], f32, tag="pR", name="pRt")
                pZt = psum.tile([SH, BC], f32, tag="pZ", name="pZt")
                pNt = psum.tile([SH, BC], f32, tag="pN", name="pNt")
                xi = xchunks[tau // XC][:, tau % XC, :] if tau < T else None
                if xi is not None:
                    nc.tensor.matmul(pRt[:], w0[:, 0:SH], xi, start=True, stop=False)
                    nc.tensor.matmul(pZt[:], w0[:, SH : 2 * SH], xi, start=True, stop=False)
                    nc.tensor.matmul(pNt[:], w0[:, 2 * SH : 3 * SH], xi, start=True, stop=False)
                return (pRt, pZt, pNt, xi is not None)

            gates = alloc_gates(0)
            pending_y = None
            for tau in range(NT):
                if tau % XC == 0 and (tau // XC) + 1 < nchunks:
                    load_xchunk(tau // XC + 1)

                pR, pZ, pN, has_x = gates
                pHN = hnpsum.tile([SH, BC], f32, tag="pHN")

                # mm order: chain consumers are r (pR), tm (pHN), v (pN),
                # then sigma_zc (pZ) which can lag until after sigma_n.
                gate_mms = {
                    "R": lambda: nc.tensor.matmul(pR[:], wr[:], prev[:], start=not has_x, stop=True),
                    "Z": lambda: nc.tensor.matmul(pZ[:], wz[:], prev[:], start=not has_x, stop=True),
                    "N": lambda: nc.tensor.matmul(pN[:], wn[:], prev[:], start=not has_x, stop=True),
                    "H": lambda: nc.tensor.matmul(pHN[:], whn[:], prev[:], start=True, stop=True),
                }
                for gk in _MM_ORDER:
                    gate_mms[gk]()

                # prefetch next tick's x-side matmuls into the other psum bank
                if tau + 1 < NT:
                    gates_next = alloc_gates(tau + 1)
                else:
                    gates_next = None

                r = work.tile([SH, BC], f32, tag="r")
                zc = work.tile([SH, BC], f32, tag="zc")
                tm = work.tile([SH, BC], f32, tag="tm")
                v = work.tile([SH, BC], f32, tag="v")
                m = work.tile([SH, BC], f32, tag="m")

                def act_r():
                    nc.scalar.activation(r[:], pR[:], SIG)

                def act_zc():
                    nc.scalar.activation(zc[:], pZ[:], SIG)

                def act_n():
                    nc.vector.tensor_mul(tm[:], r[:], pHN[:])
                    nc.vector.tensor_add(v[:], tm[:], pN[:])
                    nc.scalar.activation(m[:], v[:], SIG)

                acts = {"r": act_r, "z": act_zc, "n": act_n}
                for ak in _ACT_ORDER:
                    acts[ak]()

                # wt = (zc - 1) * g_prev  (= -(1-z)*g)
                wt = work.tile([SH, BC], f32, tag="wt")
                nc.vector.scalar_tensor_tensor(wt[:], zc[:], 1.0, prev[0:SH, :], SUB, MUL)
                u = work.tile([SH, BC], f32, tag="u")
                nc.vector.tensor_mul(u[:], zc[:], m[:])

                gnew = gpool.tile([SA, BC], f32, tag="g")
                gsub_inst = nc.vector.tensor_sub(gnew[0:SH, :], u[:], wt[:])
                if tau < 4:
                    l = tau + 1  # layer l state must be 0.5 before its first valid tick
                    nc.sync.dma_start(
                        out=gnew[OFFS[l] : OFFS[l] + H[l], :], in_=dRST[0 : H[l], :]
                    )

                handles[tau] = gnew
                prev = gnew
                gates = gates_next

                # flush the previous tick-group's y psum: emitted AFTER this
                # tick's chain DVE ops so the copy queues behind them (its
                # wait would otherwise park the engine FIFO ahead of the chain)
                if pending_y is not None and tau - pending_y[2] >= 3:
                    yp_p, t0_p, _ = pending_y
                    ysb = ypool.tile([D, YC, BC], f32, tag="ysb")
                    # order the copy AFTER this tick's state update on DVE so
                    # its sem wait can't park the engine queue ahead of the
                    # chain ops; by now the y matmuls have long completed
                    ci = nc.vector.tensor_copy(ysb[:], yp_p[:])
                    add_dep_helper(ci.ins, gsub_inst.ins, sync=False,
                                   reason="defer y-copy behind chain")
                    nc.sync.dma_start(
                        out=YT[t0_p : t0_p + YC].rearrange("t f b -> f t b"),
                        in_=ysb[:],
                    )
                    pending_y = None

                # output: project h4 for ticks tau-3..tau -> timesteps tau-7..tau-4
                if tau >= YC + 3 and (tau + 1) % YC == 0:
                    yp = ypsum.tile([D, YC * BC], f32, tag="yp")
                    for k in range(YC):
                        nc.tensor.matmul(
                            yp[:, k * BC : (k + 1) * BC],
                            wy[:],
                            handles[tau - (YC - 1) + k][:],
                            start=True,
                            stop=True,
                        )
                    pending_y = (yp, tau - (2 * YC - 1), tau)
                    for k in range(YC):
                        handles.pop(tau - (2 * YC - 1) + k, None)

            # final flush
            if pending_y is not None:
                yp_p, t0_p, _ = pending_y
                ysb = ypool.tile([D, YC, BC], f32, tag="ysb")
                if _YCOPY == "vector":
                    nc.vector.tensor_copy(ysb[:], yp_p[:])
                else:
                    nc.scalar.copy(ysb[:], yp_p[:])
                nc.sync.dma_start(
                    out=YT[t0_p : t0_p + YC].rearrange("t f b -> f t b"),
                    in_=ysb[:],
                )

    _split_excess_waits(nc)
    _prog_cache[T] = nc
    return nc


def _run(X_full, weights, T):
    """X_full: [BZ, T, D] float32. Returns [BZ, T, D]."""
    from concourse.bass_utils import run_bass_kernel_spmd

    nc = _build_program(T)
    in_maps = []
    for c in range(NCORES):
        xs = X_full[c * BC : (c + 1) * BC]  # [BC, T, D]
        XTc = np.ascontiguousarray(xs.transpose(1, 2, 0))  # [T, D, BC]
        in_maps.append({"XT": XTc, **weights})
    res = run_bass_kernel_spmd(nc, in_maps, core_ids=list(range(NCORES)))
    outs = []
    for c in range(NCORES):
        YTc = res.results[c]["YT"]  # [T, D, BC]
        outs.append(np.ascontiguousarray(YTc.transpose(2, 0, 1)))
    return np.concatenate(outs, 0)


def kernel(**inputs):
    X = np.asarray(inputs["imputed_X"], np.float32)
    weights = _build_weights(inputs)
    return _run(X, weights, X.shape[1])
